# revision 19
# baseline (speedup 1.0000x reference)
"""Area-attention (pykt-style, MAX_AREA_WIDTH=3) Trainium2 kernel.

Strategy (v2)
-------------
Pure data-parallel over (batch, head): B*H = 64 pairs, 8 pairs per core,
core c gets batch c.  No collectives.

Device work per (b, h) pair (L=512, D=64, W=3) is reduced to the three
O(L^2)-ish stages only -- QK^T, exp, PV -- everything O(L*D) lives on
the host:

  * Q arrives transposed and duplicated on both partition halves
    ([128, 512] bf16); K window-means arrive as two channels:
    k01 = [kT | ks2/2] packed on halves, k2 = ks3/3 duplicated.
    QK^T runs as two concurrent 64-row "lanes" (disjoint PE row groups):
    lane0 = s0 + s2(m0,m3), lane1 = s1 + s2(m1,m2) -- balanced 1920
    stream-cycles per lane, with LDWEIGHTS overlapping the other lane.
  * Scores are computed TRANSPOSED: S^T[j, q] (j on partitions), so the
    softmax numerator P^T = exp(S^T)*mask is already in the layout the
    PV matmul contraction needs.  Causality: area row j of segment s is
    visible to queries q >= j+s; fully-masked 128-wide q-blocks are
    skipped.
  * exp() is batched (PSUM [128, 3, 512] tiles, one ACTIVATE per
    q-group) on the Activation engine -- the critical resource
    (3840 elem/lane/pair at 1 elem/cycle/lane @1.2 GHz ~= 27 us/core).
    An early dummy exp pre-loads the ACT table during input DMA.
  * Diagonal-block masks: a single host-built [128, 2, 3, 128] bf16
    constant multiplied over pt tiles on DVE (one op per q-group, plus
    one for the packed m=3 block) -- all-SBUF bf16 so DVE perf mode
    applies.
  * V windows arrive pre-expanded as vse [128, 3, 4, 65] (t = 128a + p),
    SUM-windows with a 65th all-ones column, so O_ext^T = vse^T @ P^T
    accumulates softmax denominators as row 64 for free.
  * Device output is the raw transposed O_ext^T [65, 512] fp32 per pair,
    copied out in column chunks as each becomes final ([0:128) after
    PV(g0), [128:256) after PV(g1), rest after PV(g2)) so the output DMA
    overlaps the remaining accumulation.  The host does the final
    divide-by-denominator, the [d, q] -> [q, d] transpose, and the
    zero_pad row-0 patch (exact colsum(v_area)/1533) -- all O(L*D).
  * Dummy matmuls on a zeroed tile warm the PE p-state during the input
    DMA; input DMAs ride the sync queue while vse/masks/outputs ride the
    (otherwise idle) gpsimd queue.

Measured: ~52.8 us HW exec (prior 53.2), rel err 1.5e-3.  v4 deltas:
one consolidated input DMA per pair (qd|k01|k2 packed), vse on the sync
queue, 6-matmul PE warmup fed by a vector-engine memset, and a 2-chunk
output epilogue (fewer DMA issues + DVE copies).

Dead end explored and rejected (sessions notes): an algebraic "2-channel"
scheme (Fa=exp(S0/16); E0=Fa^2, E1=Fa[j]*Fa[j+1], E2 direct) cuts ACT
work 3->2 units but requires a +1 partition shift, which TRN2 cannot do
cheaply: engine APs must be partition-aligned, SBUF->SBUF DMA runs on a
single DMA engine (~24 GB/s), and a PE permutation matmul forces
PSUM-operand DVE ops at 1x with zero free PSUM banks.
Roughly 15.5 us of that is fixed framework preamble/teardown and ~6 us
DMA-pipe startup latency; the exp stream itself is ACT-bound at ~29 us
(3840 elem/lane/pair at 1 elem/cycle @1.2 GHz, gapless mid-run).

HW gotcha found this session: two matmuls on disjoint PE row groups
execute CONCURRENTLY, so they must never write the same PSUM bank --
doing so hangs the device (CoreSim does not model the race).
"""

import numpy as np
import ml_dtypes

B, H, L, D = 8, 8, 512, 64
W = 3
NCORES = 8
HPC = (B * H) // NCORES  # (b,h) pairs per core (= H: core c takes batch c)
LP = 1533                # 512 + 511 + 510 area rows
BF16 = ml_dtypes.bfloat16

_CACHE = {}

# Results of the last device run (for test harnesses): BassKernelResults
LAST_RESULTS = None


def _numpy_reference(q, k, v, d_k, mask, zero_pad):
    """Direct numpy port of the jax reference (fallback for non-standard
    inputs; not used on the standard setup_inputs() problem)."""
    q = np.asarray(q, np.float32)
    k = np.asarray(k, np.float32)
    v = np.asarray(v, np.float32)
    mask = np.asarray(mask)
    b, h, l, d = q.shape

    def window_vals(val, merge):
        csum = np.concatenate(
            [np.zeros((b, h, 1, d), np.float32), np.cumsum(val, axis=2)], axis=2)
        parts = []
        for i in range(W):
            w = i + 1
            s = csum[:, :, w:, :] - csum[:, :, :l - w + 1, :]
            if merge == "mean":
                s = s / np.float32(w)
            parts.append(s)
        return np.concatenate(parts, axis=2)

    k_area = window_vals(k, "mean")
    v_area = window_vals(v, "sum")
    m = np.concatenate([mask[:, :, :, i:] for i in range(W)], axis=-1)
    if int(zero_pad):
        m = m.copy()
        m[:, :, 0, :] = 0
    scores = np.einsum("bhqd,bhkd->bhqk", q, k_area) / np.sqrt(
        np.float32(float(d_k)))
    scores = np.where(m == 0, np.float32(-1e32), scores)
    scores = scores - scores.max(axis=-1, keepdims=True)
    e = np.exp(scores)
    attn = e / e.sum(axis=-1, keepdims=True)
    return np.einsum("bhqk,bhkd->bhqd", attn, v_area).astype(np.float32)


def _is_standard(q, k, v, d_k, mask, zero_pad):
    if q.shape != (B, H, L, D) or k.shape != q.shape or v.shape != q.shape:
        return False
    if int(d_k) != D or int(zero_pad) != 1:
        return False
    tril = np.tril(np.ones((L, L), mask.dtype))
    return bool((np.asarray(mask) == tril).all())


def _build_graph():
    """Builds the single-core Bass/Tile graph (identical on all 8 cores)."""
    import concourse.mybir as mybir
    import concourse.tile as tile
    from concourse import bacc

    fp32 = mybir.dt.float32
    bf16 = mybir.dt.bfloat16

    nc = bacc.Bacc()
    # qd | k01 | k2 packed in one dram tensor -> one input DMA per pair
    in0_d = nc.declare_dram_parameter("in0", [HPC, 128, 3 * L], bf16,
                                      isOutput=False)
    vse_d = nc.declare_dram_parameter("vse", [HPC, 128, W, 4, D + 1], bf16,
                                      isOutput=False)
    dm2_d = nc.declare_dram_parameter("dm2", [128, 3, W, 128], bf16,
                                      isOutput=False)
    out_d = nc.declare_dram_parameter("out", [HPC, D + 1, L], fp32,
                                      isOutput=True)

    # q-groups: list of (m, qb_offset); m covers q in [128m, 512).
    # Group 2 packs m=2 (qb 0,1) and m=3 (qb 2) into the same tiles.
    GROUPS = [[(0, 0)], [(1, 0)], [(2, 0), (3, 2)]]
    GNQ = [4, 3, 3]  # 128-wide q-blocks per group tile

    # segment -> PE lane (row half).  s0 always lane0 (kT on k01 rows
    # 0:64), s1 always lane1 (ks2 on k01 rows 64:128), s2 alternates by
    # GROUP (ks3 duplicated on both halves).  Two matmuls on disjoint row
    # groups execute concurrently, so they must never write the same PSUM
    # bank -- segment s is bank s of the group's ps tile, hence all of a
    # group's s2 matmuls share one lane.  Balance: lane0 = 1792, lane1 =
    # 2048 stream-cycles per pair.
    S2LANE = [0, 64, 64]

    def lane_of(s, g):
        if s == 0:
            return 0
        if s == 1:
            return 64
        return S2LANE[g]

    with tile.TileContext(nc) as tc:
        with (
            tc.tile_pool(name="const", bufs=1) as constp,
            tc.tile_pool(name="inp", bufs=3) as inp,
            tc.tile_pool(name="ptp", bufs=6) as ptp,
            tc.tile_pool(name="outp", bufs=2) as outp,
            tc.tile_pool(name="psS", bufs=2, space="PSUM") as psS,
            tc.tile_pool(name="psO", bufs=1, space="PSUM") as psO,
            tc.tile_pool(name="psF", bufs=1, space="PSUM") as psF,
        ):
            import os
            if not os.environ.get("AA_NO_WARM"):
                # ---- ACT exp-table warm-up (no data deps; loads the Exp
                # table during the initial input DMA) ----
                warm = constp.tile([1, 2], bf16)
                nc.vector.memset(warm[:], 0.0)
                nc.scalar.activation(
                    warm[0:1, 1:2], warm[0:1, 0:1],
                    mybir.ActivationFunctionType.Exp, scale=1.0)

            # ---- PE p-state warm-up: the Tensor engine ramps to full
            # clock only after ~3us of continuous execution; run dummy
            # matmuls on a zeroed tile during the initial input DMA so the
            # first real QK runs at full speed ----
            # HAM flips to K=8/8 only after a ~3.4us fully-busy PE window;
            # a short warm-up starts the density and small "filler" matmuls
            # (emitted at the cold-window seams where the PE queue head
            # would stall) keep it dense until the flip.  All dummy work
            # targets a dedicated PSUM bank so it never WAR-blocks real QK.
            wb = constp.tile([64, 512], bf16)
            nc.vector.memset(wb[:], 0.0)
            fillt = psF.tile([128, 512], fp32, tag="psF", name="fill")

            def emit_fill(n, free=256):
                for _ in range(n):
                    nc.tensor.matmul(
                        fillt[:, 0:free], lhsT=wb[:, 0:128],
                        rhs=wb[:, 0:free], start=True, stop=True)

            if not os.environ.get("AA_NO_PEWARM"):
                nwarm = int(os.environ.get("AA_PEWARM_N", "4"))
                emit_fill(nwarm, free=512)

            # ---- diag-block mask constant ----
            dm2 = constp.tile([128, 3, W, 128], bf16)
            if os.environ.get("AA_DEV_MASK"):
                # build on device (gpsimd) instead of DMA from host
                Alu = mybir.AluOpType
                nc.vector.memset(dm2[:], 1.0)
                nc.gpsimd.affine_select(
                    out=dm2[:], in_=dm2[:],
                    compare_op=Alu.is_ge, fill=0.0,
                    base=0, channel_multiplier=-1,
                    pattern=[[128, 2], [-1, W], [1, 128]])
            else:
                nc.gpsimd.dma_start(dm2[:], dm2_d[:])

            state = {}

            def emit_dma(h):
                if h == 0:
                    # split pair-0's input so QK(0) s0/s1 can start before
                    # the k2 plane lands (~0.4us earlier first exp)
                    in0 = inp.tile([128, 2 * L], bf16, tag="in0a",
                                   name="in0a")
                    nc.sync.dma_start(in0[:], in0_d[0, :, 0:2 * L])
                    in0b = inp.tile([128, L], bf16, tag="in0b", name="in0b")
                    nc.sync.dma_start(in0b[:], in0_d[0, :, 2 * L:3 * L])
                else:
                    in0 = inp.tile([128, 3 * L], bf16, tag="in0", name="in0")
                    nc.sync.dma_start(in0[:], in0_d[h])
                    in0b = None
                vse = inp.tile([128, W, 4, D + 1], bf16, tag="vse", name="vse")
                nc.sync.dma_start(vse[:], vse_d[h])
                state[h] = {"in0": in0, "in0b": in0b, "vse": vse,
                            "ps": {}, "pt": {}}

            def emit_qk(h, g):
                st = state[h]
                in0 = st["in0"]
                ps = psS.tile([128, W, 512], fp32, tag="psS", name="ps")
                st["ps"][g] = ps
                for s in range(W):
                    r = lane_of(s, g)
                    kb = L if s < 2 else 2 * L  # k01 | k2 column base
                    for (m, qb) in GROUPS[g]:
                        q0 = 128 * m
                        if s == 2 and st["in0b"] is not None:
                            lhsT = st["in0b"][r:r + 64, q0:q0 + 128]
                        else:
                            lhsT = in0[r:r + 64, kb + q0:kb + q0 + 128]
                        nc.tensor.matmul(
                            ps[:, s, 128 * qb:128 * qb + 512 - q0],
                            lhsT=lhsT,
                            rhs=in0[r:r + 64, q0:512],
                            start=True, stop=True)

            def emit_exp(h, g):
                nq = GNQ[g]
                st = state[h]
                ps = st["ps"][g]
                pt = ptp.tile([128, 4, W, 128], bf16, tag="pt", name="pt")
                st["pt"][g] = pt
                nc.scalar.activation(
                    pt[:, 0:nq].rearrange("p b s w -> p s b w"),
                    ps[:, :, 0:128 * nq].rearrange("p s (b w) -> p s b w",
                                                   w=128),
                    mybir.ActivationFunctionType.Exp,
                    scale=float(1.0 / np.sqrt(D)))
                # diagonal-block (and off-diagonal corner) masks: one
                # multiply per group (dm2 planes: diag, corner, diag --
                # plane 2 handles g2's packed m=3 block at qb 2)
                hi = 3 if g == 2 else 2
                nc.vector.tensor_mul(pt[:, 0:hi], pt[:, 0:hi], dm2[:, 0:hi])

            def emit_pv(h, g, only_m=None):
                # PV matmuls all contract over the full 128 rows, so they
                # serialize on the PE -- any emission order is race-free.
                # m-then-s for g2 lets the m=3 mask multiply overlap m=2's
                # matmuls (only_m selects one m-block for chunked output).
                st = state[h]
                if g == 0:
                    st["oT"] = psO.tile([D + 1, 512], fp32, tag="psO",
                                        name="oT_ps")
                oT_ps = st["oT"]
                vse = st["vse"]
                pt = st["pt"][g]
                for (m, qb) in GROUPS[g]:
                    if only_m is not None and m != only_m:
                        continue
                    q0 = 128 * m
                    for s in range(W):
                        first = (g == 0 and s == 0 and m == 0)
                        last = (g == 2 and s == W - 1 and m == 3)
                        nc.tensor.matmul(
                            oT_ps[:, q0:512],
                            lhsT=vse[:, s, m, :],
                            rhs=pt[:, qb:qb + 4 - m, s, :],
                            start=first, stop=last)

            # oT columns [0:128) are final after PV(g0) (only m=0 matmuls
            # touch them), [128:256) after PV(g1), [256:384) after g2's
            # m=2 matmuls, [384:512) after m=3: copy+DMA each chunk as
            # soon as it is final.  (CoreSim's accumulation-group read
            # check cannot express per-region closure; AA_BIGCOPY falls
            # back to one copy after PV(g2).)
            BIGCOPY = bool(os.environ.get("AA_BIGCOPY"))

            def emit_epi_chunk(h, c0, c1, last=False):
                st = state[h]
                if BIGCOPY:
                    if last:
                        oc = outp.tile([D + 1, 512], fp32, tag="oc",
                                       name="oc")
                        nc.vector.tensor_copy(oc[:], st["oT"][:])
                        nc.gpsimd.dma_start(out_d[h], oc[:])
                        state.pop(h)
                    return
                oc = outp.tile([D + 1, c1 - c0], fp32, tag=f"oc{c0}",
                               name=f"oc{c0}")
                nc.vector.tensor_copy(oc[:], st["oT"][:, c0:c1])
                nc.gpsimd.dma_start(out_d[h, :, c0:c1], oc[:])
                if last:
                    state.pop(h)

            # Group-granular software pipeline.  Per iteration (pair h):
            # ACT streams exp(h,g1), exp(h,g2), exp(h+1,g0) continuously;
            # PE fills with QK of those groups then PV(h, g0..g2).
            emit_dma(0)
            emit_dma(1)
            emit_qk(0, 0)
            emit_exp(0, 0)
            for h in range(HPC):
                if h + 2 < HPC:
                    emit_dma(h + 2)
                emit_qk(h, 1)
                emit_exp(h, 1)
                emit_qk(h, 2)
                if h + 1 < HPC:
                    emit_exp(h, 2)
                    emit_qk(h + 1, 0)
                    emit_exp(h + 1, 0)
                    if h <= 2:
                        # cold-window PE density for the HAM (PE would
                        # otherwise idle here until exp(h,g2) completes)
                        emit_fill(4)
                    emit_pv(h, 0)
                    emit_pv(h, 1)
                    emit_epi_chunk(h, 0, 256)
                    emit_pv(h, 2)
                    emit_epi_chunk(h, 256, 512, last=True)
                else:
                    # last pair: run PV(g0,g1) concurrently with exp(g2) so
                    # only PV(g2) + one copy trail the final ACTIVATE
                    emit_pv(h, 0)
                    emit_pv(h, 1)
                    emit_epi_chunk(h, 0, 256)
                    emit_exp(h, 2)
                    emit_pv(h, 2, only_m=2)
                    emit_epi_chunk(h, 256, 384)
                    emit_pv(h, 2, only_m=3)
                    emit_epi_chunk(h, 384, 512, last=True)

    nc.finalize()
    return nc


def _host_prep(q, k, v):
    """Transpose/expand/cast/shard the inputs. Returns per-core in_maps."""
    q = np.asarray(q, np.float32)
    k = np.asarray(k, np.float32)
    v = np.asarray(v, np.float32)

    # kT / ks2 (mean of 2, /2 folded) / ks3 (mean of 3, /3 folded),
    # each [B, H, 64, L].  Tail entries past the last valid window are
    # phantom areas -- always causally masked -- so any finite value is
    # fine; reuse the shorter-window values.
    kT = np.ascontiguousarray(k.transpose(0, 1, 3, 2))
    ks2 = np.zeros_like(kT)
    ks3 = np.zeros_like(kT)
    ks2[..., :L - 1] = (kT[..., :L - 1] + kT[..., 1:]) * 0.5
    ks2[..., L - 1] = kT[..., L - 1]
    ks3[..., :L - 2] = (kT[..., :L - 2] + kT[..., 1:L - 1] + kT[..., 2:]) / 3.0
    ks3[..., L - 2:] = ks2[..., L - 2:]

    # in0 = [qd | k01 | k2]: qd = q^T duplicated on both halves; k01 = kT on
    # rows 0:64 + ks2 on 64:128; k2 = ks3 duplicated.  One DMA per pair.
    in0 = np.empty((B, H, 128, 3 * L), np.float32)
    qT = q.transpose(0, 1, 3, 2)
    in0[:, :, 0:D, 0:L] = qT
    in0[:, :, D:2 * D, 0:L] = qT
    in0[:, :, 0:D, L:2 * L] = kT
    in0[:, :, D:2 * D, L:2 * L] = ks2
    in0[:, :, 0:D, 2 * L:3 * L] = ks3
    in0[:, :, D:2 * D, 2 * L:3 * L] = ks3
    in0 = in0.astype(BF16)

    # vse[b, h, p, s, a, 0:64] = sum_{u<=s} v[b, h, 128a+p+u, :] (0 past L-s)
    # vse[..., 64] = 1.0 (accumulates softmax denominators as oT row 64)
    vse = np.zeros((B, H, W, L, D + 1), np.float32)
    vse[..., D] = 1.0
    acc = v.copy()
    for s in range(W):
        if s > 0:
            acc = acc[:, :, :L - s, :] + v[:, :, s:, :]
        vse[:, :, s, :L - s, :D] = acc
    vse = np.ascontiguousarray(
        vse.reshape(B, H, W, 4, 128, D + 1).transpose(0, 1, 4, 2, 3, 5)
    ).astype(BF16)

    # diag-block mask constant dm2[p, b, s, w]:
    #   b=0 (diagonal block): keep iff w >= p + s
    #   b=1 (first off-diagonal block): keep iff 128 + w >= p + s
    #       (masks only (p=127, s=2, w=0))
    pp = np.arange(128)[:, None, None, None]
    bb = np.arange(2)[None, :, None, None]
    ss = np.arange(W)[None, None, :, None]
    ww = np.arange(128)[None, None, None, :]
    dm2 = ((128 * bb + ww - pp - ss) >= 0).astype(BF16)
    dm2 = np.concatenate([dm2, dm2[:, 0:1]], axis=1)  # plane 2 = diag again

    in_maps = []
    for c in range(NCORES):
        in_maps.append({
            "in0": np.ascontiguousarray(in0[c]),
            "vse": np.ascontiguousarray(vse[c]),
            "dm2": dm2,
        })
    return in_maps


def _host_epilogue(oT, v):
    """oT: [B, HPC, 65, 512] per-core stacked -> full [B, H, L, D] output.

    Divides numerator rows by the denominator row, transposes [d, q] ->
    [q, d], and patches the zero_pad row 0 with the exact uniform mean
    of v_area (softmax over a fully-masked row is uniform)."""
    v = np.asarray(v, np.float32)
    num = oT[:, :, 0:D, :]            # [B, H, D, L]
    den = oT[:, :, D:D + 1, :]        # [B, H, 1, L]
    out = np.ascontiguousarray(
        (num / den).transpose(0, 1, 3, 2)).astype(np.float32)

    # colsum(v_area) = 6*S - 3*v[0] - v[1] - 3*v[-1] - v[-2] where S=sum(v)
    S = v.sum(axis=2)
    colsum = (6.0 * S - 3.0 * v[:, :, 0] - v[:, :, 1]
              - 3.0 * v[:, :, -1] - v[:, :, -2])
    out[:, :, 0, :] = colsum / np.float32(LP)
    return out


def _ensure_ntff_hook():
    """The agent image's antenv package lacks axon_hooks; synthesize it and
    register the ctypes NTFF profile hook so trace=True yields exec_time_ns."""
    import sys
    import types
    try:
        import antenv.axon_hooks  # noqa: F401
        return
    except ImportError:
        pass
    mod = types.ModuleType("antenv.axon_hooks")
    mod._hook = None

    def set_axon_ntff_profile_hook(h):
        mod._hook = h

    def get_axon_ntff_profile_hook():
        return mod._hook

    mod.set_axon_ntff_profile_hook = set_axon_ntff_profile_hook
    mod.get_axon_ntff_profile_hook = get_axon_ntff_profile_hook
    sys.modules["antenv.axon_hooks"] = mod
    try:
        import antenv
        antenv.axon_hooks = mod
    except ImportError:
        pass
    try:
        from trn_agent_boot.trn_boot import _ntff_profile_via_ctypes
        hook = _ntff_profile_via_ctypes("/opt/axon/libaxon_pjrt.so")
        if hook is not None:
            mod._hook = hook
    except Exception:
        pass


def _run_device(in_maps, trace=False):
    import concourse.bass_utils as bass_utils

    if "nc" not in _CACHE:
        _CACHE["nc"] = _build_graph()
    nc = _CACHE["nc"]

    if trace:
        _ensure_ntff_hook()
        # No artifact bucket in this container; skip the S3-ish upload.
        if not getattr(bass_utils.upload_artifacts, "_patched", False):
            def _no_upload(tmpdir):
                return tmpdir
            _no_upload._patched = True
            bass_utils.upload_artifacts = _no_upload
        try:
            res = bass_utils.run_bass_kernel_spmd(
                nc, in_maps, core_ids=list(range(NCORES)), trace=True)
        except Exception as e:  # fall back to an untraced run
            print(f"trace run failed ({type(e).__name__}: {e}); retrying untraced")
            res = bass_utils.run_bass_kernel_spmd(
                nc, in_maps, core_ids=list(range(NCORES)), trace=False)
    else:
        res = bass_utils.run_bass_kernel_spmd(
            nc, in_maps, core_ids=list(range(NCORES)), trace=False)
    global LAST_RESULTS
    LAST_RESULTS = res
    return res


def kernel(q, k, v, d_k, mask, zero_pad):
    import os
    if not _is_standard(q, k, v, d_k, mask, zero_pad):
        return _numpy_reference(q, k, v, d_k, mask, zero_pad)

    in_maps = _host_prep(q, k, v)
    trace = bool(os.environ.get("AREA_ATTN_TRACE"))
    res = _run_device(in_maps, trace=trace)
    oT = np.stack([np.asarray(res.results[c]["out"]) for c in range(NCORES)])
    return _host_epilogue(oT.astype(np.float32), v)



# revision 20
# speedup vs baseline: 1.0476x; 1.0476x over previous
"""Area-attention (pykt-style, MAX_AREA_WIDTH=3) Trainium2 kernel.

Strategy (v2)
-------------
Pure data-parallel over (batch, head): B*H = 64 pairs, 8 pairs per core,
core c gets batch c.  No collectives.

Device work per (b, h) pair (L=512, D=64, W=3) is reduced to the three
O(L^2)-ish stages only -- QK^T, exp, PV -- everything O(L*D) lives on
the host:

  * Q arrives transposed and duplicated on both partition halves
    ([128, 512] bf16); K window-means arrive as two channels:
    k01 = [kT | ks2/2] packed on halves, k2 = ks3/3 duplicated.
    QK^T runs as two concurrent 64-row "lanes" (disjoint PE row groups):
    lane0 = s0 + s2(m0,m3), lane1 = s1 + s2(m1,m2) -- balanced 1920
    stream-cycles per lane, with LDWEIGHTS overlapping the other lane.
  * Scores are computed TRANSPOSED: S^T[j, q] (j on partitions), so the
    softmax numerator P^T = exp(S^T)*mask is already in the layout the
    PV matmul contraction needs.  Causality: area row j of segment s is
    visible to queries q >= j+s; fully-masked 128-wide q-blocks are
    skipped.
  * exp() is batched (PSUM [128, 3, 512] tiles, one ACTIVATE per
    q-group) on the Activation engine -- the critical resource
    (3840 elem/lane/pair at 1 elem/cycle/lane @1.2 GHz ~= 27 us/core).
    An early dummy exp pre-loads the ACT table during input DMA.
  * Diagonal-block masks: a single host-built [128, 2, 3, 128] bf16
    constant multiplied over pt tiles on DVE (one op per q-group, plus
    one for the packed m=3 block) -- all-SBUF bf16 so DVE perf mode
    applies.
  * V windows arrive pre-expanded as vse [128, 3, 4, 65] (t = 128a + p),
    SUM-windows with a 65th all-ones column, so O_ext^T = vse^T @ P^T
    accumulates softmax denominators as row 64 for free.
  * Device output is the raw transposed O_ext^T [65, 512] fp32 per pair,
    copied out in column chunks as each becomes final ([0:128) after
    PV(g0), [128:256) after PV(g1), rest after PV(g2)) so the output DMA
    overlaps the remaining accumulation.  The host does the final
    divide-by-denominator, the [d, q] -> [q, d] transpose, and the
    zero_pad row-0 patch (exact colsum(v_area)/1533) -- all O(L*D).
  * Dummy matmuls on a zeroed tile warm the PE p-state during the input
    DMA; input DMAs ride the sync queue while vse/masks/outputs ride the
    (otherwise idle) gpsimd queue.

Measured: ~52.8 us HW exec (prior 53.2), rel err 1.5e-3.  v4 deltas:
one consolidated input DMA per pair (qd|k01|k2 packed), vse on the sync
queue, 6-matmul PE warmup fed by a vector-engine memset, and a 2-chunk
output epilogue (fewer DMA issues + DVE copies).

Dead end explored and rejected (sessions notes): an algebraic "2-channel"
scheme (Fa=exp(S0/16); E0=Fa^2, E1=Fa[j]*Fa[j+1], E2 direct) cuts ACT
work 3->2 units but requires a +1 partition shift, which TRN2 cannot do
cheaply: engine APs must be partition-aligned, SBUF->SBUF DMA runs on a
single DMA engine (~24 GB/s), and a PE permutation matmul forces
PSUM-operand DVE ops at 1x with zero free PSUM banks.
Roughly 15.5 us of that is fixed framework preamble/teardown and ~6 us
DMA-pipe startup latency; the exp stream itself is ACT-bound at ~29 us
(3840 elem/lane/pair at 1 elem/cycle @1.2 GHz, gapless mid-run).

HW gotcha found this session: two matmuls on disjoint PE row groups
execute CONCURRENTLY, so they must never write the same PSUM bank --
doing so hangs the device (CoreSim does not model the race).
"""

import numpy as np
import ml_dtypes

B, H, L, D = 8, 8, 512, 64
W = 3
NCORES = 8
HPC = (B * H) // NCORES  # (b,h) pairs per core (= H: core c takes batch c)
LP = 1533                # 512 + 511 + 510 area rows
BF16 = ml_dtypes.bfloat16

_CACHE = {}

# Results of the last device run (for test harnesses): BassKernelResults
LAST_RESULTS = None


def _numpy_reference(q, k, v, d_k, mask, zero_pad):
    """Direct numpy port of the jax reference (fallback for non-standard
    inputs; not used on the standard setup_inputs() problem)."""
    q = np.asarray(q, np.float32)
    k = np.asarray(k, np.float32)
    v = np.asarray(v, np.float32)
    mask = np.asarray(mask)
    b, h, l, d = q.shape

    def window_vals(val, merge):
        csum = np.concatenate(
            [np.zeros((b, h, 1, d), np.float32), np.cumsum(val, axis=2)], axis=2)
        parts = []
        for i in range(W):
            w = i + 1
            s = csum[:, :, w:, :] - csum[:, :, :l - w + 1, :]
            if merge == "mean":
                s = s / np.float32(w)
            parts.append(s)
        return np.concatenate(parts, axis=2)

    k_area = window_vals(k, "mean")
    v_area = window_vals(v, "sum")
    m = np.concatenate([mask[:, :, :, i:] for i in range(W)], axis=-1)
    if int(zero_pad):
        m = m.copy()
        m[:, :, 0, :] = 0
    scores = np.einsum("bhqd,bhkd->bhqk", q, k_area) / np.sqrt(
        np.float32(float(d_k)))
    scores = np.where(m == 0, np.float32(-1e32), scores)
    scores = scores - scores.max(axis=-1, keepdims=True)
    e = np.exp(scores)
    attn = e / e.sum(axis=-1, keepdims=True)
    return np.einsum("bhqk,bhkd->bhqd", attn, v_area).astype(np.float32)


def _is_standard(q, k, v, d_k, mask, zero_pad):
    if q.shape != (B, H, L, D) or k.shape != q.shape or v.shape != q.shape:
        return False
    if int(d_k) != D or int(zero_pad) != 1:
        return False
    tril = np.tril(np.ones((L, L), mask.dtype))
    return bool((np.asarray(mask) == tril).all())


def _build_graph():
    """Builds the single-core Bass/Tile graph (identical on all 8 cores)."""
    import concourse.mybir as mybir
    import concourse.tile as tile
    from concourse import bacc

    fp32 = mybir.dt.float32
    bf16 = mybir.dt.bfloat16

    nc = bacc.Bacc()
    # qd | k01 | k2 packed in one dram tensor -> one input DMA per pair
    in0_d = nc.declare_dram_parameter("in0", [HPC, 128, 3 * L], bf16,
                                      isOutput=False)
    vse_d = nc.declare_dram_parameter("vse", [HPC, 128, W, 4, D + 1], bf16,
                                      isOutput=False)
    dm2_d = nc.declare_dram_parameter("dm2", [128, 2, W, 128], bf16,
                                      isOutput=False)
    out_d = nc.declare_dram_parameter("out", [HPC, D + 1, L], fp32,
                                      isOutput=True)

    # q-groups: list of (m, qb_offset); m covers q in [128m, 512).
    # Group 2 packs m=2 (qb 0,1) and m=3 (qb 2) into the same tiles.
    GROUPS = [[(0, 0)], [(1, 0)], [(2, 0), (3, 2)]]
    GNQ = [4, 3, 3]  # 128-wide q-blocks per group tile

    # segment -> PE lane (row half).  s0 always lane0 (kT on k01 rows
    # 0:64), s1 always lane1 (ks2 on k01 rows 64:128), s2 alternates by
    # GROUP (ks3 duplicated on both halves).  Two matmuls on disjoint row
    # groups execute concurrently, so they must never write the same PSUM
    # bank -- segment s is bank s of the group's ps tile, hence all of a
    # group's s2 matmuls share one lane.  Balance: lane0 = 1792, lane1 =
    # 2048 stream-cycles per pair.
    S2LANE = [0, 64, 64]

    def lane_of(s, g):
        if s == 0:
            return 0
        if s == 1:
            return 64
        return S2LANE[g]

    with tile.TileContext(nc) as tc:
        with (
            tc.tile_pool(name="const", bufs=1) as constp,
            tc.tile_pool(name="inp", bufs=3) as inp,
            tc.tile_pool(name="ptp", bufs=6) as ptp,
            tc.tile_pool(name="outp", bufs=2) as outp,
            tc.tile_pool(name="psS", bufs=2, space="PSUM") as psS,
            tc.tile_pool(name="psO", bufs=1, space="PSUM") as psO,
            tc.tile_pool(name="psF", bufs=1, space="PSUM") as psF,
        ):
            import os
            if not os.environ.get("AA_NO_WARM"):
                # ---- ACT exp-table warm-up (no data deps; loads the Exp
                # table during the initial input DMA) ----
                warm = constp.tile([1, 2], bf16)
                nc.vector.memset(warm[:], 0.0)
                nc.scalar.activation(
                    warm[0:1, 1:2], warm[0:1, 0:1],
                    mybir.ActivationFunctionType.Exp, scale=1.0)

            # ---- PE p-state warm-up: the Tensor engine ramps to full
            # clock only after ~3us of continuous execution; run dummy
            # matmuls on a zeroed tile during the initial input DMA so the
            # first real QK runs at full speed ----
            # HAM flips to K=8/8 only after a ~3.4us fully-busy PE window;
            # a short warm-up starts the density and small "filler" matmuls
            # (emitted at the cold-window seams where the PE queue head
            # would stall) keep it dense until the flip.  All dummy work
            # targets a dedicated PSUM bank so it never WAR-blocks real QK.
            wb = constp.tile([64, 512], bf16)
            nc.vector.memset(wb[:], 0.0)
            fillt = psF.tile([128, 512], fp32, tag="psF", name="fill")

            def emit_fill(n, free=256):
                for _ in range(n):
                    nc.tensor.matmul(
                        fillt[:, 0:free], lhsT=wb[:, 0:128],
                        rhs=wb[:, 0:free], start=True, stop=True)

            if not os.environ.get("AA_NO_PEWARM"):
                nwarm = int(os.environ.get("AA_PEWARM_N", "4"))
                emit_fill(nwarm, free=512)

            # ---- diag-block mask constant ----
            dm2 = constp.tile([128, 2, W, 128], bf16)
            if os.environ.get("AA_DEV_MASK"):
                # build on device (gpsimd) instead of DMA from host
                Alu = mybir.AluOpType
                nc.vector.memset(dm2[:], 1.0)
                nc.gpsimd.affine_select(
                    out=dm2[:], in_=dm2[:],
                    compare_op=Alu.is_ge, fill=0.0,
                    base=0, channel_multiplier=-1,
                    pattern=[[128, 2], [-1, W], [1, 128]])
            else:
                nc.gpsimd.dma_start(dm2[:], dm2_d[:])

            state = {}

            def emit_dma(h):
                if h == 0:
                    # split pair-0's input so QK(0) s0/s1 can start before
                    # the k2 plane lands (~0.4us earlier first exp)
                    in0 = inp.tile([128, 2 * L], bf16, tag="in0a",
                                   name="in0a")
                    nc.sync.dma_start(in0[:], in0_d[0, :, 0:2 * L])
                    in0b = inp.tile([128, L], bf16, tag="in0b", name="in0b")
                    nc.sync.dma_start(in0b[:], in0_d[0, :, 2 * L:3 * L])
                else:
                    in0 = inp.tile([128, 3 * L], bf16, tag="in0", name="in0")
                    nc.sync.dma_start(in0[:], in0_d[h])
                    in0b = None
                vse = inp.tile([128, W, 4, D + 1], bf16, tag="vse", name="vse")
                nc.sync.dma_start(vse[:], vse_d[h])
                state[h] = {"in0": in0, "in0b": in0b, "vse": vse,
                            "ps": {}, "pt": {}}

            def emit_qk(h, g):
                st = state[h]
                in0 = st["in0"]
                ps = psS.tile([128, W, 512], fp32, tag="psS", name="ps")
                st["ps"][g] = ps
                for s in range(W):
                    r = lane_of(s, g)
                    kb = L if s < 2 else 2 * L  # k01 | k2 column base
                    for (m, qb) in GROUPS[g]:
                        q0 = 128 * m
                        if s == 2 and st["in0b"] is not None:
                            lhsT = st["in0b"][r:r + 64, q0:q0 + 128]
                        else:
                            lhsT = in0[r:r + 64, kb + q0:kb + q0 + 128]
                        nc.tensor.matmul(
                            ps[:, s, 128 * qb:128 * qb + 512 - q0],
                            lhsT=lhsT,
                            rhs=in0[r:r + 64, q0:512],
                            start=True, stop=True)

            def emit_exp(h, g):
                nq = GNQ[g]
                st = state[h]
                ps = st["ps"][g]
                pt = ptp.tile([128, 4, W, 128], bf16, tag="pt", name="pt")
                st["pt"][g] = pt
                nc.scalar.activation(
                    pt[:, 0:nq].rearrange("p b s w -> p s b w"),
                    ps[:, :, 0:128 * nq].rearrange("p s (b w) -> p s b w",
                                                   w=128),
                    mybir.ActivationFunctionType.Exp,
                    scale=float(1.0 / np.sqrt(D)))
                # diagonal-block (and off-diagonal corner) masks, batched:
                # one multiply per qb 0..1; group 2 adds one for m=3 at qb 2.
                nc.vector.tensor_mul(pt[:, 0:2], pt[:, 0:2], dm2[:])
                if g == 2:
                    nc.vector.tensor_mul(pt[:, 2:3], pt[:, 2:3],
                                         dm2[:, 0:1])

            def emit_pv(h, g, only_m=None):
                # PV matmuls all contract over the full 128 rows, so they
                # serialize on the PE -- any emission order is race-free.
                # m-then-s for g2 lets the m=3 mask multiply overlap m=2's
                # matmuls (only_m selects one m-block for chunked output).
                st = state[h]
                if g == 0:
                    st["oT"] = psO.tile([D + 1, 512], fp32, tag="psO",
                                        name="oT_ps")
                oT_ps = st["oT"]
                vse = st["vse"]
                pt = st["pt"][g]
                for (m, qb) in GROUPS[g]:
                    if only_m is not None and m != only_m:
                        continue
                    q0 = 128 * m
                    for s in range(W):
                        first = (g == 0 and s == 0 and m == 0)
                        last = (g == 2 and s == W - 1 and m == 3)
                        nc.tensor.matmul(
                            oT_ps[:, q0:512],
                            lhsT=vse[:, s, m, :],
                            rhs=pt[:, qb:qb + 4 - m, s, :],
                            start=first, stop=last)

            # oT columns [0:128) are final after PV(g0) (only m=0 matmuls
            # touch them), [128:256) after PV(g1), [256:384) after g2's
            # m=2 matmuls, [384:512) after m=3: copy+DMA each chunk as
            # soon as it is final.  (CoreSim's accumulation-group read
            # check cannot express per-region closure; AA_BIGCOPY falls
            # back to one copy after PV(g2).)
            BIGCOPY = bool(os.environ.get("AA_BIGCOPY"))

            def emit_epi_chunk(h, c0, c1, last=False):
                st = state[h]
                if BIGCOPY:
                    if last:
                        oc = outp.tile([D + 1, 512], fp32, tag="oc",
                                       name="oc")
                        nc.vector.tensor_copy(oc[:], st["oT"][:])
                        nc.gpsimd.dma_start(out_d[h], oc[:])
                        state.pop(h)
                    return
                oc = outp.tile([D + 1, c1 - c0], fp32, tag=f"oc{c0}",
                               name=f"oc{c0}")
                nc.vector.tensor_copy(oc[:], st["oT"][:, c0:c1])
                nc.gpsimd.dma_start(out_d[h, :, c0:c1], oc[:])
                if last:
                    state.pop(h)

            # Group-granular software pipeline.  Per iteration (pair h):
            # ACT streams exp(h,g1), exp(h,g2), exp(h+1,g0) continuously;
            # PE fills with QK of those groups then PV(h, g0..g2).
            emit_dma(0)
            emit_dma(1)
            emit_qk(0, 0)
            emit_exp(0, 0)
            for h in range(HPC):
                if h + 2 < HPC:
                    emit_dma(h + 2)
                emit_qk(h, 1)
                emit_exp(h, 1)
                emit_qk(h, 2)
                if h + 1 < HPC:
                    emit_exp(h, 2)
                    emit_qk(h + 1, 0)
                    emit_exp(h + 1, 0)
                    if h <= 2:
                        # cold-window PE density for the HAM (PE would
                        # otherwise idle here until exp(h,g2) completes)
                        emit_fill(4)
                    emit_pv(h, 0)
                    emit_pv(h, 1)
                    emit_epi_chunk(h, 0, 256)
                    emit_pv(h, 2)
                    emit_epi_chunk(h, 256, 512, last=True)
                else:
                    # last pair: run PV(g0,g1) concurrently with exp(g2) so
                    # only PV(g2) + one copy trail the final ACTIVATE
                    emit_pv(h, 0)
                    emit_pv(h, 1)
                    emit_epi_chunk(h, 0, 256)
                    emit_exp(h, 2)
                    emit_pv(h, 2)
                    emit_epi_chunk(h, 256, 512, last=True)

    nc.finalize()
    return nc


def _host_prep(q, k, v):
    """Transpose/expand/cast/shard the inputs. Returns per-core in_maps."""
    q = np.asarray(q, np.float32)
    k = np.asarray(k, np.float32)
    v = np.asarray(v, np.float32)

    # kT / ks2 (mean of 2, /2 folded) / ks3 (mean of 3, /3 folded),
    # each [B, H, 64, L].  Tail entries past the last valid window are
    # phantom areas -- always causally masked -- so any finite value is
    # fine; reuse the shorter-window values.
    kT = np.ascontiguousarray(k.transpose(0, 1, 3, 2))
    ks2 = np.zeros_like(kT)
    ks3 = np.zeros_like(kT)
    ks2[..., :L - 1] = (kT[..., :L - 1] + kT[..., 1:]) * 0.5
    ks2[..., L - 1] = kT[..., L - 1]
    ks3[..., :L - 2] = (kT[..., :L - 2] + kT[..., 1:L - 1] + kT[..., 2:]) / 3.0
    ks3[..., L - 2:] = ks2[..., L - 2:]

    # in0 = [qd | k01 | k2]: qd = q^T duplicated on both halves; k01 = kT on
    # rows 0:64 + ks2 on 64:128; k2 = ks3 duplicated.  One DMA per pair.
    in0 = np.empty((B, H, 128, 3 * L), np.float32)
    qT = q.transpose(0, 1, 3, 2)
    in0[:, :, 0:D, 0:L] = qT
    in0[:, :, D:2 * D, 0:L] = qT
    in0[:, :, 0:D, L:2 * L] = kT
    in0[:, :, D:2 * D, L:2 * L] = ks2
    in0[:, :, 0:D, 2 * L:3 * L] = ks3
    in0[:, :, D:2 * D, 2 * L:3 * L] = ks3
    in0 = in0.astype(BF16)

    # vse[b, h, p, s, a, 0:64] = sum_{u<=s} v[b, h, 128a+p+u, :] (0 past L-s)
    # vse[..., 64] = 1.0 (accumulates softmax denominators as oT row 64)
    vse = np.zeros((B, H, W, L, D + 1), np.float32)
    vse[..., D] = 1.0
    acc = v.copy()
    for s in range(W):
        if s > 0:
            acc = acc[:, :, :L - s, :] + v[:, :, s:, :]
        vse[:, :, s, :L - s, :D] = acc
    vse = np.ascontiguousarray(
        vse.reshape(B, H, W, 4, 128, D + 1).transpose(0, 1, 4, 2, 3, 5)
    ).astype(BF16)

    # diag-block mask constant dm2[p, b, s, w]:
    #   b=0 (diagonal block): keep iff w >= p + s
    #   b=1 (first off-diagonal block): keep iff 128 + w >= p + s
    #       (masks only (p=127, s=2, w=0))
    pp = np.arange(128)[:, None, None, None]
    bb = np.arange(2)[None, :, None, None]
    ss = np.arange(W)[None, None, :, None]
    ww = np.arange(128)[None, None, None, :]
    dm2 = ((128 * bb + ww - pp - ss) >= 0).astype(BF16)

    in_maps = []
    for c in range(NCORES):
        in_maps.append({
            "in0": np.ascontiguousarray(in0[c]),
            "vse": np.ascontiguousarray(vse[c]),
            "dm2": dm2,
        })
    return in_maps


def _host_epilogue(oT, v):
    """oT: [B, HPC, 65, 512] per-core stacked -> full [B, H, L, D] output.

    Divides numerator rows by the denominator row, transposes [d, q] ->
    [q, d], and patches the zero_pad row 0 with the exact uniform mean
    of v_area (softmax over a fully-masked row is uniform)."""
    v = np.asarray(v, np.float32)
    num = oT[:, :, 0:D, :]            # [B, H, D, L]
    den = oT[:, :, D:D + 1, :]        # [B, H, 1, L]
    out = np.ascontiguousarray(
        (num / den).transpose(0, 1, 3, 2)).astype(np.float32)

    # colsum(v_area) = 6*S - 3*v[0] - v[1] - 3*v[-1] - v[-2] where S=sum(v)
    S = v.sum(axis=2)
    colsum = (6.0 * S - 3.0 * v[:, :, 0] - v[:, :, 1]
              - 3.0 * v[:, :, -1] - v[:, :, -2])
    out[:, :, 0, :] = colsum / np.float32(LP)
    return out


def _ensure_ntff_hook():
    """The agent image's antenv package lacks axon_hooks; synthesize it and
    register the ctypes NTFF profile hook so trace=True yields exec_time_ns."""
    import sys
    import types
    try:
        import antenv.axon_hooks  # noqa: F401
        return
    except ImportError:
        pass
    mod = types.ModuleType("antenv.axon_hooks")
    mod._hook = None

    def set_axon_ntff_profile_hook(h):
        mod._hook = h

    def get_axon_ntff_profile_hook():
        return mod._hook

    mod.set_axon_ntff_profile_hook = set_axon_ntff_profile_hook
    mod.get_axon_ntff_profile_hook = get_axon_ntff_profile_hook
    sys.modules["antenv.axon_hooks"] = mod
    try:
        import antenv
        antenv.axon_hooks = mod
    except ImportError:
        pass
    try:
        from trn_agent_boot.trn_boot import _ntff_profile_via_ctypes
        hook = _ntff_profile_via_ctypes("/opt/axon/libaxon_pjrt.so")
        if hook is not None:
            mod._hook = hook
    except Exception:
        pass


def _run_device(in_maps, trace=False):
    import concourse.bass_utils as bass_utils

    if "nc" not in _CACHE:
        _CACHE["nc"] = _build_graph()
    nc = _CACHE["nc"]

    if trace:
        _ensure_ntff_hook()
        # No artifact bucket in this container; skip the S3-ish upload.
        if not getattr(bass_utils.upload_artifacts, "_patched", False):
            def _no_upload(tmpdir):
                return tmpdir
            _no_upload._patched = True
            bass_utils.upload_artifacts = _no_upload
        try:
            res = bass_utils.run_bass_kernel_spmd(
                nc, in_maps, core_ids=list(range(NCORES)), trace=True)
        except Exception as e:  # fall back to an untraced run
            print(f"trace run failed ({type(e).__name__}: {e}); retrying untraced")
            res = bass_utils.run_bass_kernel_spmd(
                nc, in_maps, core_ids=list(range(NCORES)), trace=False)
    else:
        res = bass_utils.run_bass_kernel_spmd(
            nc, in_maps, core_ids=list(range(NCORES)), trace=False)
    global LAST_RESULTS
    LAST_RESULTS = res
    return res


def kernel(q, k, v, d_k, mask, zero_pad):
    import os
    if not _is_standard(q, k, v, d_k, mask, zero_pad):
        return _numpy_reference(q, k, v, d_k, mask, zero_pad)

    in_maps = _host_prep(q, k, v)
    trace = bool(os.environ.get("AREA_ATTN_TRACE"))
    res = _run_device(in_maps, trace=trace)
    oT = np.stack([np.asarray(res.results[c]["out"]) for c in range(NCORES)])
    return _host_epilogue(oT.astype(np.float32), v)

